# revision 6
# baseline (speedup 1.0000x reference)
"""Trainium2 Bass kernel for nn_Depth_CA (depth-coded-aperture Wiener pipeline).

Strategy
--------
Every fft/ifft+shift combo in the reference is a constant 256x256 complex
matrix sandwich Y = A @ X @ A.T (all four transform matrices F@P, P@G,
P@F@P, P@G@P satisfy B == A.T).  On the PE array each sandwich is two
matmul groups with the DATA as the stationary operand and host-precomputed
constants [ATr|ATi], [-ATi|ATr] as 512-wide moving operands; PSUM
accumulation implements the complex arithmetic, so no transposes and no
vector-engine combine work are needed:

    MM1: PSUM = X^T @ A^T   (= (A X)^T)          X      stationary
    MM2: PSUM = (A X) @ A^T (= A X A^T)          (AX)^T stationary

Matmuls run in float32r (TF32-class, ~1.4e-4/MM, full PE rate at free
dim >= 256); rounding to f32r rides on the PSUM-drain copies.

Sharding: depths padded 15->16, 2 per core across 8 cores.  Each core
computes its 6 (depth, band) PSF units, the 12 image/result FFTs
(replicated), the blur and Wiener stages for its own depths, with one
AllReduce(add) for the depth-summed `result` and one AllReduce(max) for
the final normalization.  The mid-pipeline result/max(result) provably
cancels and is skipped.
"""
import os
import sys

for _p in ("/opt/trn_rl_repo", os.path.expanduser("~/.axon_site/_ro/trn_rl_repo")):
    if os.path.isdir(_p) and _p not in sys.path:
        sys.path.insert(0, _p)

import numpy as np

N = 256
ND, NB, B = 15, 3, 4
NDP = 16               # padded depth count
NCORES = 8
DPC = NDP // NCORES    # depths per core = 2

# ---------------------------------------------------------------- host constants
def _host_constants():
    ZI, Z0, RADII, PX = 0.05, 2.5, 0.002, 6.22e-6
    F_ = 1.0 / (1.0 / ZI + 1.0 / Z0)
    L_SEN = PX * N
    L_LEN = 2 * RADII * 2
    LAMB = np.array([460.0, 550.0, 640.0]) * 1e-9

    def deta(l_um):
        l = np.asarray(l_um, dtype=np.float64)
        return (1.5375 + 0.00829045 * l**-2 - 0.000211046 * l**-4) - 1.0

    R_ = F_ * deta(5.5e-7 * 1e6)
    FLMB = R_ / deta(LAMB * 1e6)
    ZS = np.sort(-3 * np.log(np.linspace(0.9, 11, ND)) + 8)
    DU = L_LEN / N
    u = np.arange(-L_LEN / 2, L_LEN / 2, DU)
    X_, Y_ = np.meshgrid(u, u)
    XY = X_ * X_ + Y_ * Y_
    RAD = (np.sqrt(XY) <= RADII).astype(np.float64)
    fx1 = np.fft.fftshift(np.arange(-1 / (2 * DU), 1 / (2 * DU), 1 / L_LEN))
    FX1, FY1 = np.meshgrid(fx1, fx1)
    FF = FX1 * FX1 + FY1 * FY1

    K_ = 2 * np.pi / LAMB
    COEF = (-K_ / (2 * FLMB[0]))[None, :] + K_[None, :] / (2 * ZS[:, None]) \
        + (np.pi * (L_LEN - L_SEN) / (LAMB * ZI * L_LEN))[None, :]
    PHASE1 = (COEF[:, :, None, None] * XY[None, None]).astype(np.float32)
    PHASE2 = ((np.pi * LAMB * ZI * L_LEN / L_SEN)[None, :, None, None]
              * FF[None, None]).astype(np.float32)

    W1 = RAD[None, None] * np.exp(1j * PHASE1.astype(np.float64))    # (15,3,N,N)
    W2 = np.exp(-1j * PHASE2.astype(np.float64)[0])                  # (3,N,N)

    j = np.arange(N)
    F = np.exp(-2j * np.pi * np.outer(j, j) / N)
    G = np.conj(F) / N
    P = np.zeros((N, N))
    P[j, (j + N // 2) % N] = 1.0
    A1 = F @ P
    A2 = P @ G
    Fc = P @ F @ P
    Gc = P @ G @ P
    return W1, W2, (A1, A2, Fc, Gc)


def _pack_field(X):
    """complex (N,N) -> float32 [2, 128, 512] = per row-block [Re | Im]."""
    out = np.empty((2, 128, 512), np.float32)
    for rb in range(2):
        out[rb, :, 0:256] = X.real[rb * 128:(rb + 1) * 128, :]
        out[rb, :, 256:512] = X.imag[rb * 128:(rb + 1) * 128, :]
    return out


def _pack_moving(A):
    """constant A -> float32 [2 variants, 2 k-chunks, 128, 512] moving ops."""
    AT = A.T.copy()
    out = np.empty((2, 2, 128, 512), np.float32)
    for k in range(2):
        r = AT.real[k * 128:(k + 1) * 128, :]
        i = AT.imag[k * 128:(k + 1) * 128, :]
        out[0, k, :, 0:256] = r
        out[0, k, :, 256:512] = i
        out[1, k, :, 0:256] = -i
        out[1, k, :, 256:512] = r
    return out


_CONST_CACHE = {}


def _get_host_arrays():
    if "arr" not in _CONST_CACHE:
        W1, W2, mats = _host_constants()
        movA = np.stack([_pack_moving(A) for A in mats])         # [4,2,2,128,512]
        w2p = np.stack([_pack_field(W2[c]) for c in range(NB)])  # [3,2,128,512]
        w1p = []
        for core in range(NCORES):
            fields = []
            for dl in range(DPC):
                d = core * DPC + dl
                dd = d if d < ND else 0
                for c in range(NB):
                    fields.append(_pack_field(W1[dd, c]))
            w1p.append(np.stack(fields))                         # [6,2,128,512]
        R = np.kron(np.eye(16), np.ones((1, 16))).astype(np.float32)  # [16,256]
        _CONST_CACHE["arr"] = (movA, w2p, w1p, R)
    return _CONST_CACHE["arr"]


# ---------------------------------------------------------------- device program
A1_I, A2_I, FC_I, GC_I = 0, 1, 2, 3


def _build_program():
    import concourse.bass as bass  # noqa: F401
    import concourse.bass_isa as bass_isa
    import concourse.bacc as bacc
    import concourse.mybir as mybir
    import concourse.tile as tile

    dt = mybir.dt
    ALU = mybir.AluOpType
    ACTF = mybir.ActivationFunctionType

    nc = bacc.Bacc("TRN2", target_bir_lowering=False, debug=False,
                   num_devices=NCORES)

    movA_h, w2_h, w1p_h, R_h = _get_host_arrays()
    w1all_h = np.concatenate(w1p_h, axis=0)          # [48,2,128,512], d-major
    movA_d = nc.inline_tensor(movA_h, name="mova").ap()
    w2_d = nc.inline_tensor(w2_h, name="w2").ap()
    w1all_d = nc.inline_tensor(w1all_h, name="w1all").ap()
    img_d = nc.dram_tensor("imgf", [B * NB, 2, 128, 256], dt.float32, kind="ExternalInput").ap()
    map_d = nc.dram_tensor("mapf", [DPC, B, 2, 128, 256], dt.float32, kind="ExternalInput").ap()
    ht_d = nc.dram_tensor("ht", [16, 16], dt.float32, kind="ExternalInput").ap()
    r_d = nc.inline_tensor(R_h, name="rmat").ap()
    par_d = nc.dram_tensor("param", [1, 1], dt.float32, kind="ExternalInput").ap()
    mask_d = nc.dram_tensor("mask", [1, DPC], dt.float32, kind="ExternalInput").ap()
    out_d = nc.dram_tensor("out_recov", [DPC, NB, B, 256, 256], dt.float32, kind="ExternalOutput").ap()

    with tile.TileContext(nc) as tc:
        with (
            tc.tile_pool(name="res", bufs=1) as res,          # long-lived SBUF
            tc.tile_pool(name="wk", bufs=2) as wk,            # working tiles
            tc.tile_pool(name="ps", bufs=6, space="PSUM") as ps,
            tc.tile_pool(name="dram", bufs=1, space="DRAM") as dram,
        ):
            # ------------ load + round constants
            movr = [[[res.tile([128, 512], dt.float32r, tag=f"movr{a}_{v}_{k}",
                               name=f"movr{a}_{v}_{k}")
                      for k in range(2)] for v in range(2)] for a in range(4)]
            for a in range(4):
                for v in range(2):
                    for k in range(2):
                        stg = wk.tile([128, 512], dt.float32, tag="cstg", name=f"cstg{a}{v}{k}")
                        nc.sync.dma_start(stg[:], movA_d[a, v, k])
                        nc.vector.tensor_copy(movr[a][v][k][:], stg[:])

            w2t = [res.tile([128, 512], dt.float32, tag=f"w2t{c}_{rb}", name=f"w2t{c}_{rb}")
                   for c in range(NB) for rb in range(2)]
            for c in range(NB):
                for rb in range(2):
                    nc.sync.dma_start(w2t[c * 2 + rb][:], w2_d[c, rb])

            par1 = res.tile([1, 1], dt.float32, tag="par1", name="par1")
            nc.sync.dma_start(par1[:], par_d[:])
            par128 = res.tile([128, 1], dt.float32, tag="par128", name="par128")
            nc.gpsimd.partition_broadcast(par128[:], par1[:])
            mask1 = res.tile([1, DPC], dt.float32, tag="mask1", name="mask1")
            nc.sync.dma_start(mask1[:], mask_d[:])
            mask128 = res.tile([128, DPC], dt.float32, tag="mask128", name="mask128")
            nc.gpsimd.partition_broadcast(mask128[:], mask1[:])

            # ------------ CA = R^T @ (H @ R) (plain fp32, tiny)
            ht_t = res.tile([16, 16], dt.float32, tag="ht_t", name="ht_t")
            r_t = res.tile([16, 256], dt.float32, tag="r_t", name="r_t")
            nc.sync.dma_start(ht_t[:], ht_d[:])
            nc.sync.dma_start(r_t[:], r_d[:])
            ca_mid_ps = ps.tile([16, 256], dt.float32, tag="ps", name="ca_mid_ps")
            nc.tensor.matmul(ca_mid_ps[:], ht_t[:], r_t[:], start=True, stop=True)
            ca_mid = res.tile([16, 256], dt.float32, tag="ca_mid", name="ca_mid")
            nc.vector.tensor_copy(ca_mid[:], ca_mid_ps[:])
            ca = [res.tile([128, 256], dt.float32, tag=f"ca{mb}", name=f"ca{mb}")
                  for mb in range(2)]
            for mb in range(2):
                ca_ps = ps.tile([128, 256], dt.float32, tag="ps", name=f"ca_ps{mb}")
                nc.tensor.matmul(ca_ps[:], r_t[:, mb * 128:(mb + 1) * 128],
                                 ca_mid[:], start=True, stop=True)
                nc.vector.tensor_copy(ca[mb][:], ca_ps[:])

            # ------------ helpers
            def mm_sandwich_half(stat_tiles, a_idx, is_complex, name):
                """PSUM[mb] = S^T @ A^T for packed stationary S (2 tiles)."""
                psums = []
                for mb in range(2):
                    acc = ps.tile([128, 512], dt.float32, tag="ps", name=f"{name}_ps{mb}")
                    mms = []
                    for k in range(2):
                        mms.append((stat_tiles[k][:, mb * 128:(mb + 1) * 128],
                                    movr[a_idx][0][k][:]))
                        if is_complex:
                            mms.append((stat_tiles[k][:, 256 + mb * 128:256 + (mb + 1) * 128],
                                        movr[a_idx][1][k][:]))
                    for i, (lhsT, rhs) in enumerate(mms):
                        nc.tensor.matmul(acc[:], lhsT, rhs,
                                         start=(i == 0), stop=(i == len(mms) - 1))
                    psums.append(acc)
                return psums

            def drain_f32r(psums, name):
                out = [wk.tile([128, 512], dt.float32r, tag=name, name=f"{name}{mb}")
                       for mb in range(2)]
                for mb in range(2):
                    nc.scalar.copy(out[mb][:], psums[mb][:])
                return out

            def cmul_packed(out_tiles, x_tiles, y_tiles):
                """out = x * y elementwise complex on packed [128,512] tiles."""
                for rb in range(2):
                    xr, xi = x_tiles[rb][:, 0:256], x_tiles[rb][:, 256:512]
                    yr, yi = y_tiles[rb][:, 0:256], y_tiles[rb][:, 256:512]
                    o = out_tiles[rb]
                    t1 = wk.tile([128, 256], dt.float32, tag="cm1", name="cmt1")
                    t2 = wk.tile([128, 256], dt.float32, tag="cm2", name="cmt2")
                    nc.vector.tensor_tensor(t1[:], xr, yr, op=ALU.mult)
                    nc.vector.tensor_tensor(t2[:], xi, yi, op=ALU.mult)
                    nc.vector.tensor_tensor(o[:, 0:256], t1[:], t2[:], op=ALU.subtract)
                    nc.vector.tensor_tensor(t1[:], xr, yi, op=ALU.mult)
                    nc.vector.tensor_tensor(t2[:], xi, yr, op=ALU.mult)
                    nc.vector.tensor_tensor(o[:, 256:512], t1[:], t2[:], op=ALU.add)

            # ------------ stage 1: psf, psffr, K per (dloc, c)
            psffr_t = [res.tile([128, 512], dt.float32, tag=f"psffr{i}", name=f"psffr{i}")
                       for i in range(DPC * NB * 2)]
            kker_t = [res.tile([128, 512], dt.float32, tag=f"kker{i}", name=f"kker{i}")
                      for i in range(DPC * NB * 2)]

            pid6 = nc.gpsimd.partition_id() * (DPC * NB)
            for u in range(DPC * NB):
                c = u % NB
                w1t = [wk.tile([128, 512], dt.float32, tag="w1t", name=f"w1t{rb}")
                       for rb in range(2)]
                for rb in range(2):
                    nc.gpsimd.dma_start(w1t[rb][:], w1all_d[bass.ds(pid6 + u, 1), rb])
                ph = [wk.tile([128, 512], dt.float32r, tag="ph", name=f"ph{rb}")
                      for rb in range(2)]
                for rb in range(2):
                    nc.vector.tensor_tensor(ph[rb][:, 0:256], w1t[rb][:, 0:256], ca[rb][:], op=ALU.mult)
                    nc.vector.tensor_tensor(ph[rb][:, 256:512], w1t[rb][:, 256:512], ca[rb][:], op=ALU.mult)
                ps1 = mm_sandwich_half(ph, A1_I, True, "s1a")
                u1 = drain_f32r(ps1, "s1u1")
                ps2 = mm_sandwich_half(u1, A1_I, True, "s1b")
                vu2 = [wk.tile([128, 512], dt.float32r, tag="vu2", name=f"vu2{rb}")
                       for rb in range(2)]
                cmul_packed(vu2, ps2, [w2t[c * 2], w2t[c * 2 + 1]])
                ps3 = mm_sandwich_half(vu2, A2_I, True, "s1c")
                u3 = drain_f32r(ps3, "s1u3")
                ps4 = mm_sandwich_half(u3, A2_I, True, "s1d")
                psfu = [wk.tile([128, 256], dt.float32, tag="psfu", name=f"psfu{rb}")
                        for rb in range(2)]
                sums = wk.tile([128, 1], dt.float32, tag="sums", name="sums")
                rbs = [wk.tile([128, 1], dt.float32, tag="rbs", name=f"rbs{rb}")
                       for rb in range(2)]
                for rb in range(2):
                    t1 = wk.tile([128, 256], dt.float32, tag="cm1", name="sq1")
                    t2 = wk.tile([128, 256], dt.float32, tag="cm2", name="sq2")
                    nc.scalar.activation(t1[:], ps4[rb][:, 0:256], ACTF.Square)
                    nc.scalar.activation(t2[:], ps4[rb][:, 256:512], ACTF.Square)
                    nc.vector.tensor_tensor(psfu[rb][:], t1[:], t2[:], op=ALU.add)
                    nc.vector.tensor_reduce(rbs[rb][:], psfu[rb][:],
                                            axis=mybir.AxisListType.X, op=ALU.add)
                nc.vector.tensor_tensor(sums[:], rbs[0][:], rbs[1][:], op=ALU.add)
                tot128 = wk.tile([128, 1], dt.float32, tag="tot128", name="tot128")
                nc.gpsimd.partition_all_reduce(tot128[:], sums[:], channels=128,
                                               reduce_op=bass_isa.ReduceOp.add)
                inv128 = wk.tile([128, 1], dt.float32, tag="inv128", name="inv128")
                nc.vector.reciprocal(inv128[:], tot128[:])
                psft = [wk.tile([128, 256], dt.float32r, tag="psft", name=f"psft{rb}")
                        for rb in range(2)]
                for rb in range(2):
                    nc.vector.tensor_scalar_mul(psft[rb][:], psfu[rb][:], inv128[:])
                # psffr = Fc psf Fc
                pp1 = mm_sandwich_half(psft, FC_I, False, "pfa")
                pu1 = drain_f32r(pp1, "pfu")
                pp2 = mm_sandwich_half(pu1, FC_I, True, "pfb")
                for rb in range(2):
                    nc.scalar.copy(psffr_t[u * 2 + rb][:], pp2[rb][:])
                # psf_ifr = Gc psf Gc ; K = psf_ifr / (|psffr|^2 + param)
                pi1 = mm_sandwich_half(psft, GC_I, False, "pia")
                piu = drain_f32r(pi1, "piu")
                pi2 = mm_sandwich_half(piu, GC_I, True, "pib")
                for rb in range(2):
                    fr = psffr_t[u * 2 + rb][:, 0:256]
                    fi = psffr_t[u * 2 + rb][:, 256:512]
                    t1 = wk.tile([128, 256], dt.float32, tag="cm1", name="ab1")
                    t2 = wk.tile([128, 256], dt.float32, tag="cm2", name="ab2")
                    nc.vector.tensor_tensor(t1[:], fr, fr, op=ALU.mult)
                    nc.vector.tensor_tensor(t2[:], fi, fi, op=ALU.mult)
                    nc.vector.tensor_tensor(t1[:], t1[:], t2[:], op=ALU.add)
                    nc.vector.tensor_scalar_add(t1[:], t1[:], par128[:])
                    invp = wk.tile([128, 256], dt.float32, tag="invp", name="invp")
                    nc.vector.reciprocal(invp[:], t1[:])
                    nc.vector.tensor_tensor(kker_t[u * 2 + rb][:, 0:256],
                                            pi2[rb][:, 0:256], invp[:], op=ALU.mult)
                    nc.vector.tensor_tensor(kker_t[u * 2 + rb][:, 256:512],
                                            pi2[rb][:, 256:512], invp[:], op=ALU.mult)

            # ------------ imgft (replicated) -> DRAM
            imgft_dr = dram.tile([B * NB, 2, 128, 512], dt.float32, name="imgft_dr")
            for f in range(B * NB):
                im32r = [wk.tile([128, 256], dt.float32r, tag="im32r", name=f"im32r{k}")
                         for k in range(2)]
                for k in range(2):
                    imr = wk.tile([128, 256], dt.float32, tag="imr", name=f"imr{k}")
                    nc.sync.dma_start(imr[:], img_d[f, k])
                    nc.vector.tensor_copy(im32r[k][:], imr[:])
                ip1 = mm_sandwich_half(im32r, FC_I, False, "ifa")
                iu1 = drain_f32r(ip1, "ifu")
                ip2 = mm_sandwich_half(iu1, FC_I, True, "ifb")
                for rb in range(2):
                    sb = wk.tile([128, 512], dt.float32, tag="ifsb", name=f"ifsb{rb}")
                    nc.vector.tensor_copy(sb[:], ip2[rb][:])
                    nc.sync.dma_start(imgft_dr[f, rb], sb[:])

            # ------------ blur stage: partial result per (b,c)
            cc_in = dram.tile([B * NB, 2, 128, 256], dt.float32, name="cc_in")
            cc_out = dram.tile([B * NB, 2, 128, 256], dt.float32, name="cc_out")
            for b in range(B):
                mapt = [[wk.tile([128, 256], dt.float32, tag=f"mapt{dl}_{rb}", name=f"mapt{dl}_{rb}")
                         for rb in range(2)] for dl in range(DPC)]
                for dl in range(DPC):
                    for rb in range(2):
                        nc.sync.dma_start(mapt[dl][rb][:], map_d[dl, b, rb])
                for c in range(NB):
                    f = b * NB + c
                    imf = [wk.tile([128, 512], dt.float32, tag="imf", name=f"imf{rb}")
                           for rb in range(2)]
                    for rb in range(2):
                        nc.sync.dma_start(imf[rb][:], imgft_dr[f, rb])
                    racc = [wk.tile([128, 256], dt.float32, tag="racc", name=f"racc{rb}")
                            for rb in range(2)]
                    for rb in range(2):
                        nc.vector.memset(racc[rb][:], 0.0)
                    for dl in range(DPC):
                        u = dl * NB + c
                        bp = [wk.tile([128, 512], dt.float32r, tag="bp", name=f"bp{rb}")
                              for rb in range(2)]
                        cmul_packed(bp, imf, [psffr_t[u * 2], psffr_t[u * 2 + 1]])
                        bp1 = mm_sandwich_half(bp, GC_I, True, "bla")
                        bu1 = drain_f32r(bp1, "blu")
                        bp2 = mm_sandwich_half(bu1, GC_I, True, "blb")
                        for rb in range(2):
                            t1 = wk.tile([128, 256], dt.float32, tag="cm1", name="bm1")
                            t2 = wk.tile([128, 256], dt.float32, tag="cm2", name="bm2")
                            nc.scalar.activation(t1[:], bp2[rb][:, 0:256], ACTF.Square)
                            nc.scalar.activation(t2[:], bp2[rb][:, 256:512], ACTF.Square)
                            nc.vector.tensor_tensor(t1[:], t1[:], t2[:], op=ALU.add)
                            mag = wk.tile([128, 256], dt.float32, tag="mag", name="mag")
                            nc.scalar.activation(mag[:], t1[:], ACTF.Sqrt)
                            nc.vector.tensor_tensor(t2[:], mag[:], mapt[dl][rb][:], op=ALU.mult)
                            nc.vector.tensor_tensor(racc[rb][:], racc[rb][:], t2[:], op=ALU.add)
                    for rb in range(2):
                        nc.sync.dma_start(cc_in[f, rb], racc[rb][:])

            nc.gpsimd.collective_compute(
                "AllReduce", ALU.add,
                replica_groups=[list(range(NCORES))],
                ins=[cc_in[:]], outs=[cc_out[:]],
            )

            # ------------ wiener stage
            mag2_dr = dram.tile([DPC * NB * B, 2, 128, 256], dt.float32, name="mag2_dr")
            runmax = [res.tile([128, 1], dt.float32, tag=f"runmax{dl}", name=f"runmax{dl}")
                      for dl in range(DPC)]
            for dl in range(DPC):
                nc.vector.memset(runmax[dl][:], 0.0)
            for b in range(B):
                for c in range(NB):
                    f = b * NB + c
                    res_t = [wk.tile([128, 256], dt.float32r, tag="res_t", name=f"res_t{k}")
                             for k in range(2)]
                    for k in range(2):
                        rres = wk.tile([128, 256], dt.float32, tag="rres", name=f"rres{k}")
                        nc.sync.dma_start(rres[:], cc_out[f, k])
                        nc.vector.tensor_copy(res_t[k][:], rres[:])
                    rp1 = mm_sandwich_half(res_t, FC_I, False, "rfa")
                    ru1 = drain_f32r(rp1, "rfu")
                    rp2 = mm_sandwich_half(ru1, FC_I, True, "rfb")
                    resfr = [wk.tile([128, 512], dt.float32, tag="resfr", name=f"resfr{rb}")
                             for rb in range(2)]
                    for rb in range(2):
                        nc.vector.tensor_copy(resfr[rb][:], rp2[rb][:])
                    for dl in range(DPC):
                        u = dl * NB + c
                        wn = [wk.tile([128, 512], dt.float32r, tag="wn", name=f"wn{rb}")
                              for rb in range(2)]
                        cmul_packed(wn, [kker_t[u * 2], kker_t[u * 2 + 1]], resfr)
                        wp1 = mm_sandwich_half(wn, GC_I, True, "wna")
                        wu1 = drain_f32r(wp1, "wnu")
                        wp2 = mm_sandwich_half(wu1, GC_I, True, "wnb")
                        mi = (dl * NB + c) * B + b
                        for rb in range(2):
                            t1 = wk.tile([128, 256], dt.float32, tag="cm1", name="wm1")
                            t2 = wk.tile([128, 256], dt.float32, tag="cm2", name="wm2")
                            nc.scalar.activation(t1[:], wp2[rb][:, 0:256], ACTF.Square)
                            nc.scalar.activation(t2[:], wp2[rb][:, 256:512], ACTF.Square)
                            mag2 = wk.tile([128, 256], dt.float32, tag="mag2", name="mag2")
                            nc.vector.tensor_tensor(mag2[:], t1[:], t2[:], op=ALU.add)
                            piece = wk.tile([128, 1], dt.float32, tag="piece", name="piece")
                            nc.vector.tensor_reduce(piece[:], mag2[:],
                                                    axis=mybir.AxisListType.X, op=ALU.max)
                            nc.vector.tensor_tensor(runmax[dl][:], runmax[dl][:], piece[:], op=ALU.max)
                            nc.sync.dma_start(mag2_dr[mi, rb], mag2[:])

            # ------------ global max + final normalize
            ccm_in = dram.tile([1, 16], dt.float32, name="ccm_in")
            ccm_out = dram.tile([1, 16], dt.float32, name="ccm_out")
            mx = wk.tile([128, 1], dt.float32, tag="mx", name="mx")
            nc.vector.tensor_scalar_mul(mx[:], runmax[1][:], mask128[:, 1:2])
            nc.vector.tensor_tensor(mx[:], mx[:], runmax[0][:], op=ALU.max)
            gmx128 = wk.tile([128, 1], dt.float32, tag="gmx128", name="gmx128")
            nc.gpsimd.partition_all_reduce(gmx128[:], mx[:], channels=128,
                                           reduce_op=bass_isa.ReduceOp.max)
            ones16 = wk.tile([1, 16], dt.float32, tag="ones16", name="ones16")
            nc.vector.memset(ones16[:], 1.0)
            gmx16 = wk.tile([1, 16], dt.float32, tag="gmx16", name="gmx16")
            nc.vector.tensor_scalar_mul(gmx16[:], ones16[:], gmx128[0:1, :])
            nc.sync.dma_start(ccm_in[:], gmx16[:])
            nc.gpsimd.collective_compute(
                "AllReduce", ALU.max,
                replica_groups=[list(range(NCORES))],
                ins=[ccm_in[:]], outs=[ccm_out[:]],
            )
            gm = wk.tile([1, 1], dt.float32, tag="gm", name="gm")
            nc.sync.dma_start(gm[:], ccm_out[0:1, 0:1])
            ginv = wk.tile([1, 1], dt.float32, tag="ginv", name="ginv")
            nc.vector.reciprocal(ginv[:], gm[:])
            ginv128 = wk.tile([128, 1], dt.float32, tag="ginv128", name="ginv128")
            nc.gpsimd.partition_broadcast(ginv128[:], ginv[:])

            for dl in range(DPC):
                for c in range(NB):
                    for b in range(B):
                        mi = (dl * NB + c) * B + b
                        for rb in range(2):
                            m2 = wk.tile([128, 256], dt.float32, tag="m2", name="m2")
                            nc.sync.dma_start(m2[:], mag2_dr[mi, rb])
                            o = wk.tile([128, 256], dt.float32, tag="o", name="o")
                            nc.scalar.activation(o[:], m2[:], ACTF.Sqrt, scale=ginv128[:])
                            nc.sync.dma_start(out_d[dl, c, b, rb * 128:(rb + 1) * 128, :], o[:])

    nc.compile()
    return nc


_PROG_CACHE = {}


def _get_program():
    if "nc" not in _PROG_CACHE:
        _PROG_CACHE["nc"] = _build_program()
    return _PROG_CACHE["nc"]


# ---------------------------------------------------------------- entry point
def kernel(img, Map, H, parameter):
    from concourse.bass_utils import run_bass_kernel_spmd

    img = np.ascontiguousarray(np.asarray(img, np.float32))
    Map = np.ascontiguousarray(np.asarray(Map, np.float32))
    H = np.asarray(H, np.float32)
    parameter = np.asarray(parameter, np.float32)

    nc = _get_program()

    imgf = np.ascontiguousarray(img.transpose(0, 3, 1, 2)).reshape(B * NB, 2, 128, 256)
    mapt = np.ascontiguousarray(Map.transpose(3, 0, 1, 2))      # (15,4,256,256)
    ht = np.ascontiguousarray(H.reshape(16, 16).T)
    par = parameter.reshape(1, 1)

    in_maps = []
    for core in range(NCORES):
        mp = np.zeros((DPC, B, 2, 128, 256), np.float32)
        msk = np.zeros((1, DPC), np.float32)
        for dl in range(DPC):
            d = core * DPC + dl
            if d < ND:
                mp[dl] = mapt[d].reshape(B, 2, 128, 256)
                msk[0, dl] = 1.0
        in_maps.append({
            "imgf": imgf, "mapf": mp, "ht": ht, "param": par, "mask": msk,
        })

    res = run_bass_kernel_spmd(nc, in_maps, core_ids=list(range(NCORES)))

    out = np.empty((B, 256, 256, NB * ND), np.float32)
    for core in range(NCORES):
        rec = res.results[core]["out_recov"]        # [DPC, NB, B, 256, 256]
        for dl in range(DPC):
            d = core * DPC + dl
            if d >= ND:
                continue
            for c in range(NB):
                for b in range(B):
                    out[b, :, :, c * ND + d] = rec[dl, c, b]
    return out
